# revision 3
# baseline (speedup 1.0000x reference)
"""Pairwise squared L2 distance (retrieval KNN) on 8 TRN2 NeuronCores.

dist[i, j] = ||x_i||^2 + ||y_j||^2 - 2 * <x_i, y_j>

Sharding: rows of x are split across the 8 cores (data-parallel over n);
y is replicated. Each core computes a [1024, 8192] slab of the distance
matrix.

Memory-roofline design (rel tol 2e-2 allows 16-bit end to end):
- Single bf16 matmul for the cross term (x pre-scaled by -2 host-side,
  so PSUM accumulates -2<x,y>). bf16 runs the PE at full rate (fp16 is
  half rate) and lands ~1.7e-3 max rel err, 11x inside tolerance.
- Output stored as fp16 (~17 MB/core instead of 34), host casts back to
  fp32 after the gather. Device HBM traffic ~20 MB/core -> ~56 us
  roofline at 358 GB/s per core.
- Norm terms folded into the GEMM for 20/32 blocks: a second accumulate
  matmul with contraction-4 stationary {1, 1, xsq_hi, xsq_lo} (bf16
  hi/lo keeps the norm error ~4e-3 ulp-level) adds xsq[p] + ysq[j]
  directly in PSUM, so the epilogue is a single PSUM->SBUF fp16 copy.
  The remaining 12 blocks use the two-op epilogue (ScalarE bias-add of
  xsq, VectorE fp16 add of a host-precomputed ysq broadcast tile) to
  keep the PE under the DMA roofline. Copies/op1 are split ~21/11
  between ScalarE and VectorE so both stay at ~42 us < DMA ~56 us.
"""

import numpy as np
import ml_dtypes

import concourse.bass as bass
import concourse.mybir as mybir
import concourse.tile as tile
from concourse import bacc
from concourse.bass import ts
from concourse.bass_utils import run_bass_kernel_spmd

N, M, D = 8192, 8192, 128
NCORES = 8
SLAB = N // NCORES  # 1024 rows of x per core
P = 128  # partitions / m-chunk height
MCH = SLAB // P  # 8 m-chunks per core
NT = 512  # matmul free-dim tile (one fp32 PSUM bank)
GW = 4  # n-chunks per PSUM group (4 banks = 8 KiB/partition)
GCOLS = GW * NT  # 2048
NG = M // GCOLS  # 4 column groups
LW = 2048  # y load-chunk width
YC = M // LW  # 4 load chunks

NFOLD = 20  # blocks 0..19 fold the norms into the GEMM
YB0 = NFOLD // MCH * GCOLS  # 4096: first column covered by the ysq tile

_f32 = mybir.dt.float32
_f16 = mybir.dt.float16
_bf16 = mybir.dt.bfloat16
_IDENT = mybir.ActivationFunctionType.Identity

_bf16_np = ml_dtypes.bfloat16

# Unit ops (fold-copy or op1) on VectorE for these blocks; ScalarE else.
_DVE_UNIT = {2, 5, 8, 11, 14, 17, 20, 23, 26, 29, 31}

_compiled_nc = None


def _build():
    """Build + compile the single-core Bass program (SPMD across 8 cores)."""
    nc = bacc.Bacc(
        "TRN2",
        target_bir_lowering=False,
        debug=False,
        enable_asserts=False,
        num_devices=NCORES,
    )
    xh = nc.dram_tensor("xh", [D, SLAB], _bf16, kind="ExternalInput").ap()
    yh = nc.dram_tensor("yh", [D, M], _bf16, kind="ExternalInput").ap()
    normw = nc.dram_tensor("normw", [4, SLAB], _bf16, kind="ExternalInput").ap()
    normv = nc.dram_tensor("normv", [4, M], _bf16, kind="ExternalInput").ap()
    xsq = nc.dram_tensor("xsq", [P, MCH], _f32, kind="ExternalInput").ap()
    ysqb = nc.dram_tensor("ysqb", [P, M - YB0], _f16, kind="ExternalInput").ap()
    dist = nc.dram_tensor("dist", [SLAB, M], _f16, kind="ExternalOutput").ap()

    with tile.TileContext(nc) as tc:
        with (
            tc.tile_pool(name="consts", bufs=1) as cpool,
            tc.tile_pool(name="psum", bufs=2, space="PSUM") as pspool,
            tc.tile_pool(name="abuf", bufs=3) as apool,
            tc.tile_pool(name="obuf", bufs=4) as opool,
        ):
            # First-block inputs lead so the PE can start ASAP.
            xh_sb = cpool.tile([D, SLAB], _bf16)
            nc.sync.dma_start(xh_sb[:], xh[:])
            yh_sb = cpool.tile([D, M], _bf16)
            nc.sync.dma_start(yh_sb[:, ts(0, LW)], yh[:, ts(0, LW)])
            normw_sb = cpool.tile([4, SLAB], _bf16)
            nc.sync.dma_start(normw_sb[:], normw[:])
            normv_sb = cpool.tile([4, M], _bf16)
            nc.sync.dma_start(normv_sb[:], normv[:])
            xsq_sb = cpool.tile([P, MCH], _f32)
            nc.sync.dma_start(xsq_sb[:], xsq[:])
            for c in range(1, YC):
                nc.sync.dma_start(yh_sb[:, ts(c, LW)], yh[:, ts(c, LW)])
            ysqb_sb = cpool.tile([P, M - YB0], _f16)
            nc.sync.dma_start(ysqb_sb[:], ysqb[:])

            def emit_block(blk, mc, g):
                """One [128, 2048] output block: matmuls + epilogue + store."""
                fold = blk < NFOLD
                xh_w = xh_sb[:, ts(mc, P)]
                ps = pspool.tile([P, GCOLS], _f32, tag="ps")
                for jj in range(GW):
                    nc.tensor.matmul(
                        ps[:, ts(jj, NT)],
                        xh_w,
                        yh_sb[:, ts(g * GW + jj, NT)],
                        start=True,
                        stop=not fold,
                    )
                if fold:
                    nw = normw_sb[:, ts(mc, P)]
                    for jj in range(GW):
                        nc.tensor.matmul(
                            ps[:, ts(jj, NT)],
                            nw,
                            normv_sb[:, ts(g * GW + jj, NT)],
                            start=False,
                            stop=True,
                        )
                    ot = opool.tile([P, GCOLS], _f16, tag="ot")
                    if blk in _DVE_UNIT:
                        nc.vector.tensor_copy(ot[:], ps[:])
                    else:
                        nc.scalar.copy(ot[:], ps[:])
                else:
                    xsq_col = xsq_sb[:, mc : mc + 1]
                    a = apool.tile([P, GCOLS], _f16, tag="a")
                    if blk in _DVE_UNIT:
                        nc.vector.tensor_scalar_add(a[:], ps[:], xsq_col)
                    else:
                        nc.scalar.activation(
                            a[:], ps[:], _IDENT, bias=xsq_col, scale=1.0
                        )
                    ot = opool.tile([P, GCOLS], _f16, tag="ot")
                    nc.vector.tensor_add(
                        ot[:], a[:], ysqb_sb[:, g * GCOLS - YB0 : (g + 1) * GCOLS - YB0]
                    )
                nc.sync.dma_start(dist[ts(mc, P), ts(g, GCOLS)], ot[:])

            blk = 0
            for g in range(NG):
                for mc in range(MCH):
                    emit_block(blk, mc, g)
                    blk += 1

    nc.compile()
    return nc


def _get_nc():
    global _compiled_nc
    if _compiled_nc is None:
        _compiled_nc = _build()
    return _compiled_nc


def make_in_maps(x: np.ndarray, y: np.ndarray) -> list[dict[str, np.ndarray]]:
    x = np.asarray(x, dtype=np.float32)
    y = np.asarray(y, dtype=np.float32)
    x_sq = np.sum(x * x, axis=1, dtype=np.float32)
    y_sq = np.sum(y * y, axis=1, dtype=np.float32)

    xt2 = np.ascontiguousarray((-2.0 * x).T.astype(_bf16_np))  # [D, N]
    yt = np.ascontiguousarray(y.T.astype(_bf16_np))  # [D, M]

    # bf16 hi/lo splits of the norms for the fold matmul
    xsq_h = x_sq.astype(_bf16_np)
    xsq_l = (x_sq - xsq_h.astype(np.float32)).astype(_bf16_np)
    ysq_h = y_sq.astype(_bf16_np)
    ysq_l = (y_sq - ysq_h.astype(np.float32)).astype(_bf16_np)
    ones_m = np.ones(M, dtype=_bf16_np)
    # normv rows pair with normw rows along the contraction:
    #   k0: 1 * ysq_h[j], k1: 1 * ysq_l[j], k2: xsq_h[p] * 1, k3: xsq_l[p] * 1
    normv = np.ascontiguousarray(np.stack([ysq_h, ysq_l, ones_m, ones_m]))

    # fp16 ysq broadcast tile for the non-fold blocks (cols YB0..M)
    ysqb = np.ascontiguousarray(
        np.broadcast_to(y_sq[YB0:].astype(np.float16), (P, M - YB0))
    )

    in_maps = []
    ones_n = np.ones(SLAB, dtype=_bf16_np)
    for c in range(NCORES):
        sl = slice(c * SLAB, (c + 1) * SLAB)
        # [P, MCH]: column mc holds x_sq for rows mc*128..mc*128+127
        xsq_in = np.ascontiguousarray(x_sq[sl].reshape(MCH, P).T)
        normw = np.ascontiguousarray(
            np.stack([ones_n, ones_n, xsq_h[sl], xsq_l[sl]])
        )
        in_maps.append(
            {
                "xh": np.ascontiguousarray(xt2[:, sl]),
                "yh": yt,
                "normw": normw,
                "normv": normv,
                "xsq": xsq_in,
                "ysqb": ysqb,
            }
        )
    return in_maps


def kernel(x: np.ndarray, y: np.ndarray, **run_kwargs) -> np.ndarray:
    nc = _get_nc()
    in_maps = make_in_maps(x, y)
    res = run_bass_kernel_spmd(nc, in_maps, core_ids=list(range(NCORES)), **run_kwargs)
    out = np.concatenate(
        [res.results[c]["dist"] for c in range(NCORES)], axis=0
    ).astype(np.float32)
    if run_kwargs:
        kernel.last_results = res
    return out


# revision 4
# speedup vs baseline: 1.3121x; 1.3121x over previous
"""Pairwise squared L2 distance (retrieval KNN) on 8 TRN2 NeuronCores.

dist[i, j] = ||x_i||^2 + ||y_j||^2 - 2 * <x_i, y_j>

Sharding: rows of x are split across the 8 cores (data-parallel over n);
y is replicated. Each core computes a [1024, 8192] slab of the distance
matrix.

Memory-roofline design (rel tol 2e-2 allows 16-bit end to end):
- Single fp16 matmul for the cross term (x pre-scaled by -2 host-side,
  so PSUM = -2<x,y>). ~1e-3 max rel err, 20x inside tolerance. The PE
  streams 16-bit moving operands at 2 cycles/col, so the GEMM floor is
  ~55 us; hi/lo splits or norm-fold matmuls would blow through it.
- Output stored as fp16 (~17 MB/core instead of 34), host casts back to
  fp32 after the gather. Device HBM traffic ~20 MB/core -> ~56 us
  roofline at 358 GB/s per core.
- Epilogue split so ScalarE/VectorE both stay at ~53 us:
  op1: a = psum + x_sq[p]  (per-partition bias; ScalarE for 27 blocks,
       VectorE tensor_scalar for 5)
  op2: out = a + y_sq[j]   (VectorE fp16 tensor_tensor at 2x mode,
       4096 wide to amortize overhead)
  The y_sq broadcast tile comes from the host for cols 0:4096 (needed
  in the first ~15 us) and from GpSimd partition_broadcast for cols
  4096:8192 (needed after ~35 us, hiding the ~6 us Q7 library load).
"""

import numpy as np

import concourse.bass as bass
import concourse.mybir as mybir
import concourse.tile as tile
from concourse import bacc
from concourse.bass import ts
from concourse.bass_utils import run_bass_kernel_spmd

N, M, D = 8192, 8192, 128
NCORES = 8
SLAB = N // NCORES  # 1024 rows of x per core
P = 128  # partitions / m-chunk height
MCH = SLAB // P  # 8 m-chunks per core
NT = 512  # matmul free-dim tile (one fp32 PSUM bank)
GW = 4  # n-chunks per PSUM group (4 banks = 8 KiB/partition)
GCOLS = GW * NT  # 2048
NG = M // GCOLS  # 4 column groups
PCOLS = 2 * GCOLS  # 4096: op2/store width (two groups)
HB = M // 2  # 4096: host-provided half of the ysq broadcast tile

_f32 = mybir.dt.float32
_f16 = mybir.dt.float16
_IDENT = mybir.ActivationFunctionType.Identity

# op1 on VectorE for these block indices (of 32), ScalarE else.
_DVE_OP1 = {3, 10, 17, 24, 31}

_compiled_nc = None


def _build():
    """Build + compile the single-core Bass program (SPMD across 8 cores)."""
    nc = bacc.Bacc(
        "TRN2",
        target_bir_lowering=False,
        debug=False,
        enable_asserts=False,
        num_devices=NCORES,
    )
    xh = nc.dram_tensor("xh", [D, SLAB], _f16, kind="ExternalInput").ap()
    yh = nc.dram_tensor("yh", [D, M], _f16, kind="ExternalInput").ap()
    xsq = nc.dram_tensor("xsq", [P, MCH], _f32, kind="ExternalInput").ap()
    ysqb = nc.dram_tensor("ysqb", [P, HB], _f16, kind="ExternalInput").ap()
    ysqr = nc.dram_tensor("ysqr", [1, M - HB], _f16, kind="ExternalInput").ap()
    dist = nc.dram_tensor("dist", [SLAB, M], _f16, kind="ExternalOutput").ap()

    with tile.TileContext(nc) as tc:
        with (
            tc.tile_pool(name="consts", bufs=1) as cpool,
            tc.tile_pool(name="psum", bufs=2, space="PSUM") as pspool,
            tc.tile_pool(name="abuf", bufs=3) as apool,
            tc.tile_pool(name="obuf", bufs=4) as opool,
        ):
            # First-block inputs lead so the PE can start ASAP.
            xh_sb = cpool.tile([D, SLAB], _f16)
            nc.sync.dma_start(xh_sb[:], xh[:])
            yh_sb = cpool.tile([D, M], _f16)
            nc.sync.dma_start(yh_sb[:, ts(0, GCOLS)], yh[:, ts(0, GCOLS)])
            nc.sync.dma_start(yh_sb[:, ts(1, GCOLS)], yh[:, ts(1, GCOLS)])
            ysq_b = cpool.tile([P, M], _f16)
            nc.sync.dma_start(ysq_b[:, 0:HB], ysqb[:])
            xsq_sb = cpool.tile([P, MCH], _f32)
            nc.sync.dma_start(xsq_sb[:], xsq[:])
            ysqr_sb = cpool.tile([1, M - HB], _f16)
            nc.sync.dma_start(ysqr_sb[:], ysqr[:])
            nc.sync.dma_start(yh_sb[:, ts(2, GCOLS)], yh[:, ts(2, GCOLS)])
            nc.sync.dma_start(yh_sb[:, ts(3, GCOLS)], yh[:, ts(3, GCOLS)])

            # ysq_b[p, j] = y_sq[j] for the back half, built on GpSimd
            # (its ~6 us library load hides behind the first column pair).
            for c in range(2):
                nc.gpsimd.partition_broadcast(
                    ysq_b[:, HB + c * GCOLS : HB + (c + 1) * GCOLS],
                    ysqr_sb[0:1, ts(c, GCOLS)],
                )

            blk = 0
            for gp in range(NG // 2):
                for mc in range(MCH):
                    xh_w = xh_sb[:, ts(mc, P)]
                    xsq_col = xsq_sb[:, mc : mc + 1]
                    a4 = apool.tile([P, PCOLS], _f16, tag="a")
                    for half in range(2):
                        g = 2 * gp + half
                        ps = pspool.tile([P, GCOLS], _f32, tag="ps")
                        for jj in range(GW):
                            nc.tensor.matmul(
                                ps[:, ts(jj, NT)],
                                xh_w,
                                yh_sb[:, ts(g * GW + jj, NT)],
                                start=True,
                                stop=True,
                            )
                        # op1: a = psum + x_sq (per-partition)
                        ah = a4[:, ts(half, GCOLS)]
                        if blk in _DVE_OP1:
                            nc.vector.tensor_scalar_add(ah, ps[:], xsq_col)
                        else:
                            nc.scalar.activation(
                                ah, ps[:], _IDENT, bias=xsq_col, scale=1.0
                            )
                        blk += 1
                    # op2: out = a + y_sq over both groups at once
                    ot = opool.tile([P, PCOLS], _f16, tag="ot")
                    nc.vector.tensor_add(
                        ot[:], a4[:], ysq_b[:, ts(gp, PCOLS)]
                    )
                    nc.sync.dma_start(dist[ts(mc, P), ts(gp, PCOLS)], ot[:])

    nc.compile()
    return nc


def _get_nc():
    global _compiled_nc
    if _compiled_nc is None:
        _compiled_nc = _build()
    return _compiled_nc


def make_in_maps(x: np.ndarray, y: np.ndarray) -> list[dict[str, np.ndarray]]:
    x = np.asarray(x, dtype=np.float32)
    y = np.asarray(y, dtype=np.float32)
    x_sq = np.sum(x * x, axis=1, dtype=np.float32)
    y_sq = np.sum(y * y, axis=1, dtype=np.float32)

    xt2 = np.ascontiguousarray((-2.0 * x).T.astype(np.float16))  # [D, N]
    yt = np.ascontiguousarray(y.T.astype(np.float16))  # [D, M]
    ysq16 = y_sq.astype(np.float16)
    ysqb = np.ascontiguousarray(np.broadcast_to(ysq16[:HB], (P, HB)))
    ysqr = np.ascontiguousarray(ysq16[HB:].reshape(1, M - HB))

    in_maps = []
    for c in range(NCORES):
        sl = slice(c * SLAB, (c + 1) * SLAB)
        # [P, MCH]: column mc holds x_sq for rows mc*128..mc*128+127
        xsq_in = np.ascontiguousarray(x_sq[sl].reshape(MCH, P).T)
        in_maps.append(
            {
                "xh": np.ascontiguousarray(xt2[:, sl]),
                "yh": yt,
                "xsq": xsq_in,
                "ysqb": ysqb,
                "ysqr": ysqr,
            }
        )
    return in_maps


def kernel(x: np.ndarray, y: np.ndarray, **run_kwargs) -> np.ndarray:
    nc = _get_nc()
    in_maps = make_in_maps(x, y)
    res = run_bass_kernel_spmd(nc, in_maps, core_ids=list(range(NCORES)), **run_kwargs)
    out = np.concatenate(
        [res.results[c]["dist"] for c in range(NCORES)], axis=0
    ).astype(np.float32)
    if run_kwargs:
        kernel.last_results = res
    return out
